# revision 1
# baseline (speedup 1.0000x reference)
"""HGT (heterogeneous graph transformer, single edge type) on 8 trn2 NeuronCores.

Strategy: 1D node partition of destinations. Host sorts edges by dst and
assigns each core the edges whose dst falls in its node shard, so segment
softmax and aggregation are fully core-local. Node-level matmuls (k/q/v) are
computed redundantly on every core (full N) so no kv exchange is needed;
a single AllGather moves the hidden state between the two layers.

Per layer, per core:
  phase 1: kv[n] = h @ [Wk_eff | Wv_eff]  (PE, node-major rows in DRAM)
           q[n]  = h @ Wq_eff for local nodes
  phase 2: per 128-node window: batched indirect-DMA gather of kv rows for
           the window's edges; selection matrices S (is_equal vs iota) and
           S^T (contiguous-run range compare vs iota); q per edge via S^T
           matmul; alpha = rowsum(q*k) per head; ez = exp(alpha); agg/denom
           accumulated into PSUM via S matmuls.
  phase 3: h' = g * (gelu(agg/denom) @ Wa + ba) + (1-g) * h  for local nodes.
"""

import sys
import math
from contextlib import ExitStack

sys.path.insert(0, "/opt/trn_rl_repo")

import numpy as np

from concourse import bacc, bass, mybir
from concourse.bass_utils import run_bass_kernel_spmd
from concourse.masks import make_identity
from concourse.tile import TileContext

NCORES = 8
P = 128
C = 128
H = 4
D = 32
L = 2
OUT = 2
KV = 2 * C

f32 = mybir.dt.float32
bf16 = mybir.dt.bfloat16
i32 = mybir.dt.int32
i16 = mybir.dt.int16

# dtype knobs: table/gather dtype and edge-compute dtype
DT_TAB = f32
DT_EDGE = f32

LAST_RESULTS = None  # stash for test.py introspection
_NC_CACHE = {}  # (SH, W, T, NPAD, g, flags) -> compiled Bacc program


def _ap(base, pattern):
    """Raw access pattern on the same tensor/offset as `base`."""
    return bass.AP(base.tensor, base.offset, pattern)


def _build(SH, W, T, NPAD, g_vals, has_bkv, has_bq, has_ba, has_bfc):
    nc = bacc.Bacc("TRN2", target_bir_lowering=False)
    groups = [min(4, T - t0) for t0 in range(0, T, 4)]
    TP = T * P

    xT = nc.dram_tensor("xT", [P, NPAD], f32, kind="ExternalInput")
    hLoc0 = nc.dram_tensor("hLoc0", [P, SH], f32, kind="ExternalInput")
    srcix_d = nc.dram_tensor("srcix", [W, P, T], i32, kind="ExternalInput")
    dcol_d = nc.dram_tensor("dcol", [W, P, T], i16, kind="ExternalInput")
    r0_d = nc.dram_tensor("r0", [W, P, 1], i16, kind="ExternalInput")
    r1_d = nc.dram_tensor("r1", [W, P, 1], i16, kind="ExternalInput")
    Wkv_d = nc.dram_tensor("Wkv", [L, C, KV], f32, kind="ExternalInput")
    Wq_d = nc.dram_tensor("Wq", [L, C, C], f32, kind="ExternalInput")
    Wa_d = nc.dram_tensor("Wa", [L, C, C], f32, kind="ExternalInput")
    Wfc_d = nc.dram_tensor("Wfc", [C, OUT], f32, kind="ExternalInput")
    bkv_d = nc.dram_tensor("bkv", [L, P, KV], f32, kind="ExternalInput")
    bq_d = nc.dram_tensor("bq", [L, P, C], f32, kind="ExternalInput")
    bag_d = nc.dram_tensor("bag", [L, C, 1], f32, kind="ExternalInput")
    bfc_d = nc.dram_tensor("bfc", [P, OUT], f32, kind="ExternalInput")
    out_d = nc.dram_tensor("out", [SH, OUT], f32, kind="ExternalOutput")

    kvtab = nc.dram_tensor("kvtab", [NPAD, KV], DT_TAB)
    qloc = nc.dram_tensor("qloc", [SH, C], DT_EDGE)
    h1T = nc.dram_tensor("h1T", [P, SH], f32)
    h2T = nc.dram_tensor("h2T", [P, SH], f32)
    ag = nc.dram_tensor("ag", [NCORES, P, SH], f32, addr_space="Shared")

    AFT = mybir.ActivationFunctionType
    ALU = mybir.AluOpType

    with TileContext(nc) as tc, ExitStack() as ctx:
        cpool = ctx.enter_context(tc.tile_pool(name="consts", bufs=1))
        pbig = ctx.enter_context(tc.tile_pool(name="pbig", bufs=2))
        pmid = ctx.enter_context(tc.tile_pool(name="pmid", bufs=3))
        psml = ctx.enter_context(tc.tile_pool(name="psml", bufs=3))
        ps = ctx.enter_context(tc.tile_pool(name="ps", bufs=2, space="PSUM"))

        ident = cpool.tile([P, P], f32)
        make_identity(nc, ident[:])
        # iota128[p, j] = j ; iotaF[p, j] = j over the window's padded edges
        iota128 = cpool.tile([P, P], i16)
        nc.gpsimd.iota(iota128[:], pattern=[[1, P]], base=0, channel_multiplier=0)
        iotaF = cpool.tile([P, TP], i16)
        nc.gpsimd.iota(iotaF[:], pattern=[[1, TP]], base=0, channel_multiplier=0)

        wkv_sb = cpool.tile([P, L * KV], f32)
        wq_sb = cpool.tile([P, L * C], f32)
        wa_sb = cpool.tile([P, L * C], f32)
        wfc_sb = cpool.tile([P, OUT], f32)
        for l in range(L):
            nc.sync.dma_start(out=wkv_sb[:, l * KV:(l + 1) * KV], in_=Wkv_d[l])
            nc.sync.dma_start(out=wq_sb[:, l * C:(l + 1) * C], in_=Wq_d[l])
            nc.sync.dma_start(out=wa_sb[:, l * C:(l + 1) * C], in_=Wa_d[l])
        nc.sync.dma_start(out=wfc_sb[:], in_=Wfc_d[:])
        bkv_sb = cpool.tile([P, L * KV], f32)
        bq_sb = cpool.tile([P, L * C], f32)
        bag_sb = cpool.tile([P, L], f32)
        bfc_sb = cpool.tile([P, OUT], f32)
        if has_bkv:
            for l in range(L):
                nc.sync.dma_start(out=bkv_sb[:, l * KV:(l + 1) * KV], in_=bkv_d[l])
        if has_bq:
            for l in range(L):
                nc.sync.dma_start(out=bq_sb[:, l * C:(l + 1) * C], in_=bq_d[l])
        if has_ba:
            for l in range(L):
                nc.sync.dma_start(out=bag_sb[:, l:l + 1], in_=bag_d[l])
        if has_bfc:
            nc.sync.dma_start(out=bfc_sb[:], in_=bfc_d[:])

        for l in range(L):
            g = g_vals[l]
            # ---------------- phase 1: kv table (full N) + local q ----------
            for s in range(NCORES):
                for tl in range(W):
                    if l == 0:
                        src_ap = xT[:, s * SH + tl * P: s * SH + (tl + 1) * P]
                    else:
                        src_ap = ag[s][:, tl * P:(tl + 1) * P]
                    ht = psml.tile([P, P], f32, tag="p1h")
                    nc.sync.dma_start(out=ht[:], in_=src_ap)
                    pk = ps.tile([P, KV], f32, tag="p1")
                    nc.tensor.matmul(pk[:], lhsT=ht[:],
                                     rhs=wkv_sb[:, l * KV:(l + 1) * KV],
                                     start=True, stop=True)
                    row = (s * W + tl) * P
                    kvb = psml.tile([P, KV], DT_TAB, tag="kvb")
                    if has_bkv:
                        nc.vector.tensor_tensor(
                            out=kvb[:], in0=pk[:],
                            in1=bkv_sb[:, l * KV:(l + 1) * KV], op=ALU.add)
                    else:
                        nc.scalar.activation(out=kvb[:], in_=pk[:], func=AFT.Copy)
                    nc.sync.dma_start(out=kvtab[row:row + P, :], in_=kvb[:])
            hsrc = hLoc0 if l == 0 else h1T
            for tl in range(W):
                ht = psml.tile([P, P], f32, tag="p1h")
                nc.sync.dma_start(out=ht[:], in_=hsrc[:, tl * P:(tl + 1) * P])
                pq = ps.tile([P, C], f32, tag="p1")
                nc.tensor.matmul(pq[:], lhsT=ht[:],
                                 rhs=wq_sb[:, l * C:(l + 1) * C],
                                 start=True, stop=True)
                qb = psml.tile([P, C], DT_EDGE, tag="kvb")
                if has_bq:
                    nc.vector.tensor_tensor(
                        out=qb[:], in0=pq[:],
                        in1=bq_sb[:, l * C:(l + 1) * C], op=ALU.add)
                else:
                    nc.scalar.activation(out=qb[:], in_=pq[:], func=AFT.Copy)
                nc.sync.dma_start(out=qloc[tl * P:(tl + 1) * P, :], in_=qb[:])

            # ---------------- phase 2: edge aggregation ---------------------
            for w in range(W):
                dct = psml.tile([P, T], i16, tag="dct")
                nc.sync.dma_start(out=dct[:], in_=dcol_d[w])
                r0t = psml.tile([P, 1], i16, tag="r0t")
                nc.sync.dma_start(out=r0t[:], in_=r0_d[w])
                r1t = psml.tile([P, 1], i16, tag="r1t")
                nc.sync.dma_start(out=r1t[:], in_=r1_d[w])
                six = psml.tile([P, T], i32, tag="six")
                nc.sync.dma_start(out=six[:], in_=srcix_d[w])
                qw = psml.tile([P, C], DT_EDGE, tag="qw")
                nc.sync.dma_start(out=qw[:], in_=qloc[w * P:(w + 1) * P, :])
                kva = pbig.tile([P, T * KV], DT_TAB, tag="kva")
                for t in range(T):
                    # HW indirect DMA consumes one row-offset per partition
                    nc.gpsimd.indirect_dma_start(
                        out=kva[:, t * KV:(t + 1) * KV], out_offset=None,
                        in_=kvtab[:, :],
                        in_offset=bass.IndirectOffsetOnAxis(ap=six[:, t:t + 1], axis=0))

                # S[e, (t,n)] = (dcol[e,t] == n)       (edges on partitions)
                S = pbig.tile([P, TP], DT_EDGE, tag="S")
                nc.vector.tensor_tensor(
                    out=S[:].rearrange("p (t e) -> p t e", e=P),
                    in0=dct[:].to_broadcast([P, T, P]),
                    in1=_ap(iota128[:], [[P, P], [0, T], [1, P]]),
                    op=ALU.is_equal)
                # S^T[n, j] = (r0[n] <= j < r1[n])     (nodes on partitions)
                STl = pbig.tile([P, TP], DT_EDGE, tag="STl")
                nc.vector.tensor_tensor(
                    out=STl[:], in0=iotaF[:], in1=r1t[:].to_broadcast([P, TP]),
                    op=ALU.is_lt)
                ST = pbig.tile([P, TP], DT_EDGE, tag="ST")
                nc.vector.scalar_tensor_tensor(
                    out=ST[:], in0=iotaF[:], scalar=r0t[:], in1=STl[:],
                    op0=ALU.is_ge, op1=ALU.mult)

                ags = ps.tile([P, 132], f32, tag="agg")
                kva_v = kva[:].rearrange("p (t c) -> p t c", c=KV)
                t0 = 0
                for glen in groups:
                    psq = ps.tile([P, 512], f32, tag="psq")
                    for i in range(glen):
                        t = t0 + i
                        nc.tensor.matmul(psq[:, i * P:(i + 1) * P],
                                         lhsT=ST[:, t * P:(t + 1) * P],
                                         rhs=qw[:], start=True, stop=True)
                    qsb = pmid.tile([P, 512], DT_EDGE, tag="qsb")
                    nc.scalar.activation(out=qsb[:, :glen * P],
                                         in_=psq[:, :glen * P], func=AFT.Copy)
                    prod = pmid.tile([P, 512], DT_EDGE, tag="prod")
                    nc.vector.tensor_tensor(
                        out=prod[:, :glen * P].rearrange("p (t e) -> p t e", e=P),
                        in0=qsb[:, :glen * P].rearrange("p (t e) -> p t e", e=P),
                        in1=kva_v[:, t0:t0 + glen, 0:C],
                        op=ALU.mult)
                    alpha = pmid.tile([P, 16], f32, tag="alpha")
                    nc.vector.tensor_reduce(
                        out=alpha[:, :glen * H],
                        in_=prod[:, :glen * P].rearrange(
                            "p (t h d) -> p t h d", h=H, d=D),
                        axis=mybir.AxisListType.X, op=ALU.add)
                    msg = pmid.tile([P, 4 * 132], DT_EDGE, tag="msg")
                    msg_v = msg[:].rearrange("p (t c) -> p t c", c=132)
                    nc.scalar.activation(
                        out=msg_v[:, :glen, C:C + 4],
                        in_=alpha[:, :glen * H].rearrange("p (t h) -> p t h", h=H),
                        func=AFT.Exp)
                    nc.vector.tensor_tensor(
                        out=msg_v[:, :glen, 0:C].rearrange("p t (h d) -> p t h d", d=D),
                        in0=kva_v[:, t0:t0 + glen, C:KV].rearrange(
                            "p t (h d) -> p t h d", d=D),
                        in1=msg_v[:, :glen, C:C + 4].to_broadcast([P, glen, H, D]),
                        op=ALU.mult)
                    for i in range(glen):
                        t = t0 + i
                        nc.tensor.matmul(ags[:, 0:132],
                                         lhsT=S[:, t * P:(t + 1) * P],
                                         rhs=msg[:, i * 132:(i + 1) * 132],
                                         start=(t == 0), stop=(t == T - 1),
                                         skip_group_check=True)
                    t0 += glen

                # ------------ phase 3: window epilogue ----------------------
                den = psml.tile([P, 4], f32, tag="den")
                nc.vector.tensor_scalar_max(den[:], ags[:, C:C + 4], 1e-30)
                rec = psml.tile([P, 4], f32, tag="rec")
                nc.vector.reciprocal(rec[:], den[:])
                aggn = psml.tile([P, C], f32, tag="aggn")
                nc.vector.tensor_tensor(
                    out=aggn[:].rearrange("p (h d) -> p h d", d=D),
                    in0=ags[:, 0:C].rearrange("p (h d) -> p h d", d=D),
                    in1=rec[:].to_broadcast([P, H, D]),
                    op=ALU.mult)
                gact = psml.tile([P, C], f32, tag="gact")
                nc.scalar.activation(out=gact[:], in_=aggn[:], func=AFT.Gelu)
                gt = ps.tile([P, P], f32, tag="epi")
                nc.tensor.transpose(gt[:], gact[:], ident[:])
                gts = psml.tile([P, P], f32, tag="gts")
                nc.scalar.activation(out=gts[:], in_=gt[:], func=AFT.Copy)
                op_ = ps.tile([P, P], f32, tag="epi")
                nc.tensor.matmul(op_[:], lhsT=wa_sb[:, l * C:(l + 1) * C],
                                 rhs=gts[:], start=True, stop=True)
                hlt = psml.tile([P, P], f32, tag="hlt")
                nc.sync.dma_start(out=hlt[:], in_=hsrc[:, w * P:(w + 1) * P])
                hn = psml.tile([P, P], f32, tag="hn")
                nc.vector.scalar_tensor_tensor(
                    out=hn[:], in0=hlt[:], scalar=float(1.0 - g), in1=op_[:],
                    op0=ALU.mult, op1=ALU.add)
                if has_ba:
                    nc.vector.tensor_tensor(
                        out=hn[:], in0=hn[:],
                        in1=bag_sb[:, l:l + 1].to_broadcast([P, P]),
                        op=ALU.add)
                hdst = h1T if l == 0 else h2T
                nc.sync.dma_start(out=hdst[:, w * P:(w + 1) * P], in_=hn[:])

            if l == 0:
                nc.gpsimd.collective_compute(
                    "AllGather", ALU.bypass,
                    replica_groups=[list(range(NCORES))],
                    ins=[h1T[:]], outs=[ag[:]])

        # ---------------- final FC --------------------------------------
        for tl in range(W):
            h2t = psml.tile([P, P], f32, tag="p1h")
            nc.sync.dma_start(out=h2t[:], in_=h2T[:, tl * P:(tl + 1) * P])
            po = ps.tile([P, OUT], f32, tag="p1")
            nc.tensor.matmul(po[:], lhsT=h2t[:], rhs=wfc_sb[:],
                             start=True, stop=True)
            ob = psml.tile([P, OUT], f32, tag="ob")
            if has_bfc:
                nc.vector.tensor_tensor(
                    out=ob[:], in0=po[:], in1=bfc_sb[:], op=ALU.add)
            else:
                nc.scalar.activation(out=ob[:], in_=po[:], func=AFT.Copy)
            nc.sync.dma_start(out=out_d[tl * P:(tl + 1) * P, :], in_=ob[:])

    nc.compile()
    return nc


def _prep_host(x, edge_index, Wk, bk, Wq, bq, Wv, bv, a_rel, m_rel, p_rel,
               Wa, ba, skip, Wfc, bfc):
    N = x.shape[0]
    SH = int(math.ceil(N / NCORES / P)) * P
    W = SH // P
    NPAD = NCORES * SH

    # effective weights (fold per-head relation transforms + p_rel scaling)
    Wk_eff = np.einsum("lchd,lhde->lche", Wk.reshape(L, C, H, D),
                       a_rel, optimize=True).reshape(L, C, C)
    bk_eff = np.einsum("lhd,lhde->lhe", bk.reshape(L, H, D), a_rel).reshape(L, C)
    Wv_eff = np.einsum("lchd,lhde->lche", Wv.reshape(L, C, H, D),
                       m_rel, optimize=True).reshape(L, C, C)
    bv_eff = np.einsum("lhd,lhde->lhe", bv.reshape(L, H, D), m_rel).reshape(L, C)
    scale = (p_rel / np.sqrt(D)).astype(np.float32)  # [L, H]
    Wq_eff = (Wq.reshape(L, C, H, D) * scale[:, None, :, None]).reshape(L, C, C)
    bq_eff = (bq.reshape(L, H, D) * scale[:, :, None]).reshape(L, C)
    g_vals = [float(1.0 / (1.0 + np.exp(-skip[l]))) for l in range(L)]
    Wa_eff = np.stack([g_vals[l] * Wa[l] for l in range(L)])
    bag = np.stack([g_vals[l] * ba[l] for l in range(L)])
    Wkv = np.concatenate([Wk_eff, Wv_eff], axis=2)  # [L, C, 2C]
    bkv = np.concatenate([bk_eff, bv_eff], axis=1)  # [L, 2C]

    src = np.asarray(edge_index[0], np.int64)
    dst = np.asarray(edge_index[1], np.int64)
    core = dst // SH

    per_core = []
    T = 1
    for m in range(NCORES):
        sel = core == m
        d = (dst[sel] - m * SH).astype(np.int32)
        s_ = src[sel].astype(np.int32)
        o = np.argsort(d, kind="stable")
        d = d[o]
        s_ = s_[o]
        win = d >> 7
        cnt = np.bincount(win, minlength=W)
        if len(d):
            T = max(T, int(math.ceil(cnt.max() / P)))
        per_core.append((d, s_, win, cnt))

    xT = np.zeros([P, NPAD], np.float32)
    xT[:, :N] = np.ascontiguousarray(x.T)

    in_maps = []
    for m in range(NCORES):
        d, s_, win, cnt = per_core[m]
        src_idx = np.zeros([W, P, T], np.int32)
        dcol = np.full([W, P, T], -1, np.int16)
        # per-node contiguous run [r0, r1) within the window's padded edge list
        ncnt = np.bincount(d, minlength=SH).reshape(W, P)
        r1 = np.cumsum(ncnt, axis=1).astype(np.int16)
        r0 = (r1 - ncnt).astype(np.int16)
        if len(d):
            starts = np.zeros(W, np.int64)
            starts[1:] = np.cumsum(cnt)[:-1]
            j = np.arange(len(d)) - starts[win]
            t = (j >> 7).astype(np.int64)
            p = (j & 127).astype(np.int64)
            src_idx[win, p, t] = s_
            dcol[win, p, t] = (d & 127).astype(np.int16)
        in_maps.append({
            "xT": xT,
            "hLoc0": np.ascontiguousarray(xT[:, m * SH:(m + 1) * SH]),
            "srcix": src_idx,
            "dcol": dcol,
            "r0": np.ascontiguousarray(r0[:, :, None]),
            "r1": np.ascontiguousarray(r1[:, :, None]),
            "Wkv": np.ascontiguousarray(Wkv, dtype=np.float32),
            "Wq": np.ascontiguousarray(Wq_eff, dtype=np.float32),
            "Wa": np.ascontiguousarray(Wa_eff, dtype=np.float32),
            "Wfc": np.ascontiguousarray(Wfc, dtype=np.float32),
            "bkv": np.ascontiguousarray(
                np.broadcast_to(bkv[:, None, :], (L, P, KV)), dtype=np.float32),
            "bq": np.ascontiguousarray(
                np.broadcast_to(bq_eff[:, None, :], (L, P, C)), dtype=np.float32),
            "bag": np.ascontiguousarray(bag[:, :, None], dtype=np.float32),
            "bfc": np.ascontiguousarray(
                np.broadcast_to(bfc[None, :], (P, OUT)), dtype=np.float32),
        })

    flags = dict(
        has_bkv=bool(np.any(bkv != 0)),
        has_bq=bool(np.any(bq_eff != 0)),
        has_ba=bool(np.any(bag != 0)),
        has_bfc=bool(np.any(bfc != 0)),
    )
    return SH, W, T, NPAD, g_vals, in_maps, flags


def kernel(x, edge_index, Wk, bk, Wq, bq, Wv, bv, a_rel, m_rel, p_rel,
           Wa, ba, skip, Wfc, bfc, trace=False):
    global LAST_RESULTS
    x = np.asarray(x, np.float32)
    args = [np.asarray(a, np.float32) for a in
            (Wk, bk, Wq, bq, Wv, bv, a_rel, m_rel, p_rel, Wa, ba, skip, Wfc, bfc)]
    N = x.shape[0]

    SH, W, T, NPAD, g_vals, in_maps, flags = _prep_host(x, edge_index, *args)
    key = (SH, W, T, NPAD, tuple(g_vals), tuple(sorted(flags.items())))
    nc = _NC_CACHE.get(key)
    if nc is None:
        nc = _build(SH, W, T, NPAD, g_vals, **flags)
        _NC_CACHE[key] = nc
    try:
        res = run_bass_kernel_spmd(nc, in_maps, list(range(NCORES)), trace=trace)
    except ModuleNotFoundError:
        # NTFF profile hook unavailable in this environment
        res = run_bass_kernel_spmd(nc, in_maps, list(range(NCORES)), trace=False)
    LAST_RESULTS = res

    out = np.empty([N, OUT], np.float32)
    for m in range(NCORES):
        lo = m * SH
        hi = min(N, lo + SH)
        if hi > lo:
            out[lo:hi] = res.results[m]["out"][:hi - lo]
    return out

